# revision 10
# baseline (speedup 1.0000x reference)
"""Causal self-attention (B=2, T=2048, C=1024, nh=16) on 8 TRN2 NeuronCores.

Sharding: core c = 4*b + g handles batch b (2048 tokens) and head-group g
(4 heads).  Megatron-style: QKV rows and proj columns sharded by head group;
the proj partial sums (and b_proj) are reduced on the host (the
"all-reduce").

Per-core kernel, v4 — all matmul operands bf16 (PSUM accumulation stays
fp32); matmul N is capped at 512 by the ISA (s3d3_mm_num_elements):
  1. QKV projection kqvT[f,t] = Wl @ x_b.T per 512-token chunk. k-outputs
     land in zero-padded per-slot buffers (ktp_e/ktp_o) so QK matmuls
     contract K=128 with a full 128-partition moving operand.
  2. v tiles transposed on PE (f32) to [s,d] layout, packed next to shared
     all-ones column blocks (memset once) so the PV matmul also emits
     softmax row-sums pre-broadcast across the complement 64 partitions.
  3. per head, per 512-token chunk: S.T = kT.T@qT per s-tile pair into
     2-bank psum, one merged exp on ACT (bf16 out; trimmed region unread),
     0/1 causal mask on diagonal blocks (gpsimd), PV accumulation
     v_aug.T @ P.T -> yT + rowsum.
  4. normalize straight off PSUM: reciprocal_approx_fast (full-partition
     APs only — the custom DVE op mis-addresses partition offsets), DMA
     partition swap, bf16 multiplies into ysb.
  5. proj emitted per 512-chunk after the next chunk's QKV (keeps the PE
     fed while normalization completes); bf16 partials DMA'd out, host
     sums the 4 per-batch partials in fp32 and adds b_proj.

PSUM budget (8 banks): tag "big" 2-bank slots x2 bufs (attn ss / proj acc)
+ tag "sm" 1-bank slots x4 bufs (QKV accs, pys pair, v-transpose tp).
"""

import os
import numpy as np

B, T, C, NH, HD = 2, 2048, 1024, 16, 64
HPC = 4  # heads per core
NCORES = 8

_cache = {}


def _build_nc():
    from contextlib import ExitStack

    import concourse.bass as bass
    import concourse.tile as tile
    from concourse import bacc, mybir

    f32 = mybir.dt.float32
    bf16 = mybir.dt.bfloat16
    AF = mybir.ActivationFunctionType
    OP = mybir.AluOpType

    nc = bacc.Bacc("TRN2", target_bir_lowering=False, debug=False,
                   num_devices=NCORES)

    xt = nc.dram_tensor("xt", [C, T], bf16, kind="ExternalInput").ap()
    wkqv = nc.dram_tensor("wkqv", [C, 3 * HPC * HD], bf16,
                          kind="ExternalInput").ap()
    bkq = nc.dram_tensor("bkq", [128, 6], f32, kind="ExternalInput").ap()
    wproj = nc.dram_tensor("wproj", [HPC * HD, C], bf16,
                           kind="ExternalInput").ap()
    ident_d = nc.dram_tensor("ident", [128, 128], f32,
                             kind="ExternalInput").ap()
    amask_d = nc.dram_tensor("amask", [128, 128], bf16,
                             kind="ExternalInput").ap()
    outp = nc.dram_tensor("outp", [C, T], bf16, kind="ExternalOutput").ap()

    NCH = 4       # 512-wide t-chunks
    CHW = 512

    with tile.TileContext(nc) as tc, ExitStack() as ctx:
        sing = ctx.enter_context(tc.tile_pool(name="sing", bufs=1))
        xpool = ctx.enter_context(tc.tile_pool(name="xpool", bufs=2))
        ptp = ctx.enter_context(tc.tile_pool(name="ptp", bufs=4))
        rsp = ctx.enter_context(tc.tile_pool(name="rsp", bufs=4))
        rbp = ctx.enter_context(tc.tile_pool(name="rbp", bufs=2))
        osp = ctx.enter_context(tc.tile_pool(name="osp", bufs=3))
        ps = ctx.enter_context(tc.tile_pool(name="ps", bufs=2, space="PSUM"))

        # ---- resident SBUF tensors ----
        wk = sing.tile([128, 8, 768], bf16, name="wk")
        # q by head-pair (bf16); vT by head-pair in f32 for PE transposes
        kqv = sing.tile([128, 2, T], bf16, name="kqv")
        vtq = sing.tile([128, 2, T], f32, name="vtq")
        # zero-padded kT for full-K QK matmuls: even heads in rows 0:64 of
        # ktp_e (rows 64:128 zero), odd heads in rows 64:128 of ktp_o
        ktp_e = sing.tile([128, 2, T], bf16, name="ktp_e")
        ktp_o = sing.tile([128, 2, T], bf16, name="ktp_o")
        # vsb: 32 blocks of [v_A(64) | ones(128) | v_B(64)]
        vsb = sing.tile([128, 32 * 256], bf16, name="vsb")
        ysb = sing.tile([128, 2, T], bf16, name="ysb")
        wp = sing.tile([128, 2, C], bf16, name="wp")
        bkq_s = sing.tile([128, 6], f32, name="bkq_s")
        ident = sing.tile([128, 128], f32, name="ident")
        amask = sing.tile([128, 128], bf16, name="amask")

        nc.vector.memset(ktp_e[64:128, :, :], 0.0)
        nc.vector.memset(ktp_o[0:64, :, :], 0.0)
        # shared all-ones mid columns of every vsb block: PV matmul emits
        # rowsum broadcast over the complement 64 partitions of each yT
        vanchor = vsb[:, 64:65]
        vones = bass.AP(tensor=vanchor.tensor, offset=vanchor.offset,
                        ap=[vanchor.ap[0], [256, 32], [1, 128]])
        nc.vector.memset(vones, 1.0)

        xt_r = xt.rearrange("(kk p) t -> p kk t", p=128)
        wkqv_r = wkqv.rearrange("(kk p) f -> p kk f", p=128)
        outp_r = outp.rearrange("(a p) t -> p a t", p=128)

        qdma = [nc.scalar, nc.gpsimd, nc.sync]
        xts0 = xpool.tile([128, 8, CHW], bf16, name="xts")
        for k in range(8):
            qdma[k % 3].dma_start(xts0[:, k, :], xt_r[:, k, 0:CHW])
            qdma[(k + 1) % 3].dma_start(wk[:, k, :], wkqv_r[:, k, :])
        nc.sync.dma_start(wp, wproj.rearrange("(kk p) f -> p kk f", p=128))
        nc.scalar.dma_start(bkq_s, bkq)
        nc.scalar.dma_start(ident, ident_d)
        nc.gpsimd.dma_start(amask, amask_d)

        def v_stationary(j, h):
            """[128,128] AP: even slot -> [v_A|ones64], odd -> [ones64|v_B]."""
            hf, sl = h // 2, h % 2
            off = (j * 2 + hf) * 256 + 128 * sl
            return vsb[:, off:off + 128]

        xts_tiles = [xts0]

        def _qkv_store(m, acc, cols):
            if m < 2:  # kT: split into zero-padded buffers
                nc.vector.tensor_scalar_add(
                    out=ktp_e[0:64, m, cols], in0=acc[0:64, :],
                    scalar1=bkq_s[0:64, m:m + 1])
                nc.vector.tensor_scalar_add(
                    out=ktp_o[64:128, m, cols], in0=acc[64:128, :],
                    scalar1=bkq_s[64:128, m:m + 1])
            elif m < 4:
                nc.vector.tensor_scalar_add(
                    out=kqv[:, m - 2, cols], in0=acc,
                    scalar1=bkq_s[:, m:m + 1])
            else:
                nc.vector.tensor_scalar_add(
                    out=vtq[:, m - 4, cols], in0=acc,
                    scalar1=bkq_s[:, m:m + 1])

        def emit_qkv(n):
            # m: 0,1 -> kT (head pairs), 2,3 -> q, 4,5 -> v
            cols = slice(n * CHW, (n + 1) * CHW)
            with nc.named_scope(f"qkv{n}"):
                if n + 1 < NCH:  # prefetch next chunk while computing
                    nxt = xpool.tile([128, 8, CHW], bf16, name="xts")
                    for k in range(8):
                        qdma[k % 3].dma_start(
                            nxt[:, k, :],
                            xt_r[:, k, (n + 1) * CHW:(n + 2) * CHW])
                    xts_tiles.append(nxt)
                xts = xts_tiles[n]
                for m in range(6):
                    acc = ps.tile([128, CHW], f32, name="acc", tag="sm",
                                  bufs=4)
                    for k in range(8):
                        nc.tensor.matmul(
                            acc, wk[:, k, m * 128:(m + 1) * 128], xts[:, k, :],
                            start=(k == 0), stop=(k == 7))
                    _qkv_store(m, acc, cols)

        def emit_attn(n):
            cols = slice(n * CHW, (n + 1) * CHW)
            with nc.named_scope(f"attn{n}"):
                jmax = 4 * n + 3
                for hf in range(2):
                    pys = [
                        ps.tile([128, CHW], f32, name=f"py{sl}", tag="sm",
                                bufs=4)
                        for sl in range(2)]
                    pend = []
                    for a in range(2 * n + 2):
                        for sl in range(2):
                            h = 2 * hf + sl
                            ktp = ktp_e if sl == 0 else ktp_o
                            ss = ps.tile([128, 2, CHW], f32, name="ss",
                                         tag="big")
                            c0s = []
                            for idx in range(2):
                                j = 2 * a + idx
                                c0 = max(0, 128 * j - 512 * n)
                                c0s.append(c0)
                                nc.tensor.matmul(
                                    ss[:, idx, c0:],
                                    ktp[:, hf, j * 128:(j + 1) * 128],
                                    kqv[:, hf, n * CHW + c0:(n + 1) * CHW],
                                    start=True, stop=True)
                            if hf == 0 and a == 0 and sl == 1:
                                # v transposes for this chunk's s-tiles,
                                # hidden inside the attention PE stream
                                for t2 in range(2):
                                    jb = 4 * n + 2 * t2
                                    tp = ps.tile([128, 2, 2, 128], f32,
                                                 name="tp", tag="sm", bufs=4)
                                    for ji in range(2):
                                        for h2 in range(2):
                                            nc.tensor.transpose(
                                                tp[:, ji, h2, :],
                                                vtq[:, h2,
                                                    (jb + ji) * 128:
                                                    (jb + ji + 1) * 128],
                                                ident)
                                    for h2 in range(2):
                                        off = jb * 512 + h2 * 256
                                        anch = vsb[:, off:off + 1]
                                        dst = bass.AP(
                                            tensor=anch.tensor,
                                            offset=anch.offset,
                                            ap=[anch.ap[0], [512, 2],
                                                [192, 2], [1, 64]])
                                        nc.vector.tensor_copy(
                                            dst,
                                            tp[:, :, h2, :].rearrange(
                                                "p j (a b) -> p j a b", a=2))
                            for fn in pend:
                                fn()
                            pend = []
                            pt = ptp.tile([128, 2, CHW], bf16, name="pt")
                            # exp per idx half (full 512 cols: the c0-trimmed
                            # region holds exp(stale psum) but PV only
                            # streams [c0:]) so PV(idx0) starts sooner
                            for idx in range(2):
                                nc.scalar.activation(
                                    out=pt[:, idx, :], in_=ss[:, idx, :],
                                    func=AF.Exp)
                            for idx in range(2):
                                j = 2 * a + idx
                                c0 = c0s[idx]
                                if j >= 4 * n:  # diagonal: 0/1 mask post-exp
                                    nc.gpsimd.tensor_tensor(
                                        out=pt[:, idx, c0:c0 + 128],
                                        in0=pt[:, idx, c0:c0 + 128],
                                        in1=amask, op=OP.mult)

                            def mk_pv(a=a, c0s=c0s, pt=pt, h=h, py=pys[sl]):
                                for idx in range(2):
                                    j = 2 * a + idx
                                    c0 = c0s[idx]
                                    nc.tensor.matmul(
                                        py[:, c0:], v_stationary(j, h),
                                        pt[:, idx, c0:],
                                        start=(j == 0), stop=(j == jmax),
                                        skip_group_check=True)
                            pend.append(mk_pv)
                    for fn in pend:
                        fn()
                    # normalization straight off PSUM: head A yT at 0:64 of
                    # pys[0] (rowsum bcast at 64:128), head B yT at 64:128 of
                    # pys[1] (rowsum bcast at 0:64). reciprocal on the
                    # complement halves, DMA partition swap, multiply.
                    rtA = rsp.tile([128, CHW], f32, name="rtA")
                    rtB = rsp.tile([128, CHW], f32, name="rtB")
                    rb = rbp.tile([128, CHW], f32, name="rb")
                    # full-partition, offset-0 APs only: the custom DVE op
                    # mis-addresses partition-offset APs. Halves 0:64 of rtA
                    # and 64:128 of rtB hold junk reciprocals, never read.
                    nc.vector.reciprocal_approx_fast(out=rtA, in_=pys[0])
                    nc.sync.dma_start(rb[0:64, :], rtA[64:128, :])
                    nc.vector.reciprocal_approx_fast(out=rtB, in_=pys[1])
                    nc.sync.dma_start(rb[64:128, :], rtB[0:64, :])
                    nc.vector.tensor_tensor(
                        out=ysb[0:64, hf, cols], in0=pys[0][0:64, :],
                        in1=rb[0:64, :], op=OP.mult)
                    nc.vector.tensor_tensor(
                        out=ysb[64:128, hf, cols], in0=pys[1][64:128, :],
                        in1=rb[64:128, :], op=OP.mult)

        def emit_proj(n):
            cols = slice(n * CHW, (n + 1) * CHW)
            with nc.named_scope(f"proj{n}"):
                for op2 in range(4):
                    acc = ps.tile([128, 2, CHW], f32, name="pacc", tag="big")
                    for cc in range(2):
                        o = 2 * op2 + cc
                        for kk in range(2):
                            nc.tensor.matmul(
                                acc[:, cc, :],
                                wp[:, kk, o * 128:(o + 1) * 128],
                                ysb[:, kk, cols],
                                start=(kk == 0), stop=(kk == 1))
                    ot = osp.tile([128, 2, CHW], bf16, name="ot")
                    if op2 % 2 == 0:
                        nc.scalar.copy(out=ot, in_=acc)
                    else:
                        nc.vector.tensor_copy(ot, acc)
                    nc.gpsimd.dma_start(
                        outp_r[:, 2 * op2:2 * op2 + 2, cols], ot)

        # ---- chunk pipeline: next QKV right after attn covers norm latency
        emit_qkv(0)
        for n in range(NCH):
            emit_attn(n)
            if n + 1 < NCH:
                emit_qkv(n + 1)
            emit_proj(n)

    nc.compile()
    return nc


def _host_inputs(x, W_kqv, b_kqv, W_proj, b_proj):
    import ml_dtypes
    bf16 = ml_dtypes.bfloat16

    x = np.asarray(x, dtype=np.float32)
    W_kqv = np.asarray(W_kqv, dtype=np.float32)
    b_kqv = np.asarray(b_kqv, dtype=np.float32)
    W_proj = np.asarray(W_proj, dtype=np.float32)

    ident = np.eye(128, dtype=np.float32)
    ss, tt = np.meshgrid(np.arange(128), np.arange(128), indexing="ij")
    amask = (ss <= tt).astype(bf16)  # 0/1 multiplicative mask

    xts = [np.ascontiguousarray(x[b].T.astype(bf16)) for b in range(B)]

    in_maps = []
    for c in range(NCORES):
        b, g = c // 4, c % 4
        heads = [4 * g + i for i in range(HPC)]
        wl = np.concatenate(
            [W_kqv[h * 192:h * 192 + 64] for h in heads]
            + [W_kqv[h * 192 + 64:h * 192 + 128] * 0.125 for h in heads]
            + [W_kqv[h * 192 + 128:h * 192 + 192] for h in heads], axis=0)
        bl = np.concatenate(
            [b_kqv[h * 192:h * 192 + 64] for h in heads]
            + [b_kqv[h * 192 + 64:h * 192 + 128] * 0.125 for h in heads]
            + [b_kqv[h * 192 + 128:h * 192 + 192] for h in heads])
        in_maps.append({
            "xt": xts[b],
            "wkqv": np.ascontiguousarray(wl.T.astype(bf16)),
            "bkq": np.ascontiguousarray(
                bl.reshape(6, 128).T.astype(np.float32)),
            "wproj": np.ascontiguousarray(
                W_proj[:, 256 * g:256 * (g + 1)].T.astype(bf16)),
            "ident": ident,
            "amask": amask,
        })
    return in_maps


def kernel(x, W_kqv, b_kqv, W_proj, b_proj):
    from concourse.bass_utils import run_bass_kernel_spmd

    if "nc" not in _cache:
        _cache["nc"] = _build_nc()
    nc = _cache["nc"]

    in_maps = _host_inputs(x, W_kqv, b_kqv, W_proj, b_proj)
    trace = bool(int(os.environ.get("KERNEL_TRACE", "0")))
    r = run_bass_kernel_spmd(nc, in_maps, core_ids=list(range(NCORES)),
                             trace=trace)
    if trace:
        _cache["last_results"] = r
        print(f"HW exec time: {r.exec_time_ns} ns")

    b_proj = np.asarray(b_proj, dtype=np.float32)
    out = np.empty((B, T, C), dtype=np.float32)
    for b in range(B):
        acc = np.zeros((C, T), dtype=np.float32)
        for g in range(4):
            acc += r.results[4 * b + g]["outp"].astype(np.float32)
        out[b] = acc.T + b_proj[None, :]
    return out


# revision 11
# speedup vs baseline: 1.1094x; 1.1094x over previous
"""Causal self-attention (B=2, T=2048, C=1024, nh=16) on 8 TRN2 NeuronCores.

Sharding: core c = 4*b + g handles batch b (2048 tokens) and head-group g
(4 heads).  Megatron-style: QKV rows and proj columns sharded by head group;
the proj partial sums (and b_proj) are reduced on the host (the
"all-reduce").

Per-core kernel, v4 — all matmul operands bf16 (PSUM accumulation stays
fp32); matmul N is capped at 512 by the ISA (s3d3_mm_num_elements):
  1. QKV projection kqvT[f,t] = Wl @ x_b.T per 512-token chunk. k-outputs
     land in zero-padded per-slot buffers (ktp_e/ktp_o) so QK matmuls
     contract K=128 with a full 128-partition moving operand.
  2. v tiles transposed on PE (f32) to [s,d] layout, packed next to shared
     all-ones column blocks (memset once) so the PV matmul also emits
     softmax row-sums pre-broadcast across the complement 64 partitions.
  3. per head, per 512-token chunk: S.T = kT.T@qT per s-tile pair into
     2-bank psum, one merged exp on ACT (bf16 out; trimmed region unread),
     0/1 causal mask on diagonal blocks (gpsimd), PV accumulation
     v_aug.T @ P.T -> yT + rowsum.
  4. normalize straight off PSUM: reciprocal_approx_fast (full-partition
     APs only — the custom DVE op mis-addresses partition offsets), DMA
     partition swap, bf16 multiplies into ysb.
  5. proj emitted per 512-chunk after the next chunk's QKV (keeps the PE
     fed while normalization completes); bf16 partials DMA'd out, host
     sums the 4 per-batch partials in fp32 and adds b_proj.

PSUM budget (8 banks): tag "big" 2-bank slots x2 bufs (attn ss / proj acc)
+ tag "sm" 1-bank slots x4 bufs (QKV accs, pys pair, v-transpose tp).
"""

import os
import numpy as np

B, T, C, NH, HD = 2, 2048, 1024, 16, 64
HPC = 4  # heads per core
NCORES = 8

_cache = {}


def _build_nc():
    from contextlib import ExitStack

    import concourse.bass as bass
    import concourse.tile as tile
    from concourse import bacc, mybir

    f32 = mybir.dt.float32
    bf16 = mybir.dt.bfloat16
    AF = mybir.ActivationFunctionType
    OP = mybir.AluOpType

    nc = bacc.Bacc("TRN2", target_bir_lowering=False, debug=False,
                   num_devices=NCORES)

    xt = nc.dram_tensor("xt", [C, T], bf16, kind="ExternalInput").ap()
    wkqv = nc.dram_tensor("wkqv", [C, 3 * HPC * HD], bf16,
                          kind="ExternalInput").ap()
    bkq = nc.dram_tensor("bkq", [128, 6], f32, kind="ExternalInput").ap()
    wproj = nc.dram_tensor("wproj", [HPC * HD, C], bf16,
                           kind="ExternalInput").ap()
    ident_d = nc.dram_tensor("ident", [128, 128], f32,
                             kind="ExternalInput").ap()
    amask_d = nc.dram_tensor("amask", [128, 128], bf16,
                             kind="ExternalInput").ap()
    outp = nc.dram_tensor("outp", [C, T], bf16, kind="ExternalOutput").ap()

    NCH = 4       # 512-wide t-chunks
    CHW = 512

    with tile.TileContext(nc) as tc, ExitStack() as ctx:
        sing = ctx.enter_context(tc.tile_pool(name="sing", bufs=1))
        xpool = ctx.enter_context(tc.tile_pool(name="xpool", bufs=2))
        ptp = ctx.enter_context(tc.tile_pool(name="ptp", bufs=4))
        rsp = ctx.enter_context(tc.tile_pool(name="rsp", bufs=4))
        rbp = ctx.enter_context(tc.tile_pool(name="rbp", bufs=2))
        osp = ctx.enter_context(tc.tile_pool(name="osp", bufs=3))
        ps = ctx.enter_context(tc.tile_pool(name="ps", bufs=2, space="PSUM"))

        # ---- resident SBUF tensors ----
        wk = sing.tile([128, 8, 768], bf16, name="wk")
        # q by head-pair (bf16); vT by head-pair in f32 for PE transposes
        kqv = sing.tile([128, 2, T], bf16, name="kqv")
        vtq = sing.tile([128, 2, T], f32, name="vtq")
        # zero-padded kT for full-K QK matmuls: even heads in rows 0:64 of
        # ktp_e (rows 64:128 zero), odd heads in rows 64:128 of ktp_o
        ktp_e = sing.tile([128, 2, T], bf16, name="ktp_e")
        ktp_o = sing.tile([128, 2, T], bf16, name="ktp_o")
        # vsb: 32 blocks of [v_A(64) | ones(128) | v_B(64)]
        vsb = sing.tile([128, 32 * 256], bf16, name="vsb")
        ysb = sing.tile([128, 2, T], bf16, name="ysb")
        wp = sing.tile([128, 2, C], bf16, name="wp")
        bkq_s = sing.tile([128, 6], f32, name="bkq_s")
        ident = sing.tile([128, 128], f32, name="ident")
        amask = sing.tile([128, 128], bf16, name="amask")

        nc.vector.memset(ktp_e[64:128, :, :], 0.0)
        nc.vector.memset(ktp_o[0:64, :, :], 0.0)
        # shared all-ones mid columns of every vsb block: PV matmul emits
        # rowsum broadcast over the complement 64 partitions of each yT
        vanchor = vsb[:, 64:65]
        vones = bass.AP(tensor=vanchor.tensor, offset=vanchor.offset,
                        ap=[vanchor.ap[0], [256, 32], [1, 128]])
        nc.vector.memset(vones, 1.0)

        xt_r = xt.rearrange("(kk p) t -> p kk t", p=128)
        wkqv_r = wkqv.rearrange("(kk p) f -> p kk f", p=128)
        outp_r = outp.rearrange("(a p) t -> p a t", p=128)

        qdma = [nc.scalar, nc.gpsimd, nc.sync]
        xts0 = xpool.tile([128, 8, CHW], bf16, name="xts")
        # cold start: x chunk 0 and the m=0..2 weight columns first, so the
        # early QKV m-groups start as soon as possible; rest follows.
        for k in range(8):
            qdma[k % 3].dma_start(xts0[:, k, :], xt_r[:, k, 0:CHW])
            qdma[k % 3].dma_start(wk[:, k, 0:384], wkqv_r[:, k, 0:384])
        nc.scalar.dma_start(bkq_s, bkq)
        nc.gpsimd.dma_start(ident, ident_d)
        nc.sync.dma_start(amask, amask_d)
        for k in range(8):
            qdma[k % 3].dma_start(wk[:, k, 384:768], wkqv_r[:, k, 384:768])
        nc.sync.dma_start(wp, wproj.rearrange("(kk p) f -> p kk f", p=128))

        def v_stationary(j, h):
            """[128,128] AP: even slot -> [v_A|ones64], odd -> [ones64|v_B]."""
            hf, sl = h // 2, h % 2
            off = (j * 2 + hf) * 256 + 128 * sl
            return vsb[:, off:off + 128]

        xts_tiles = [xts0]

        def _qkv_store(m, acc, cols):
            if m < 2:  # kT: split into zero-padded buffers
                nc.vector.tensor_scalar_add(
                    out=ktp_e[0:64, m, cols], in0=acc[0:64, :],
                    scalar1=bkq_s[0:64, m:m + 1])
                nc.vector.tensor_scalar_add(
                    out=ktp_o[64:128, m, cols], in0=acc[64:128, :],
                    scalar1=bkq_s[64:128, m:m + 1])
            elif m < 4:
                nc.vector.tensor_scalar_add(
                    out=kqv[:, m - 2, cols], in0=acc,
                    scalar1=bkq_s[:, m:m + 1])
            else:
                nc.vector.tensor_scalar_add(
                    out=vtq[:, m - 4, cols], in0=acc,
                    scalar1=bkq_s[:, m:m + 1])

        def emit_qkv(n):
            # m: 0,1 -> kT (head pairs), 2,3 -> q, 4,5 -> v
            cols = slice(n * CHW, (n + 1) * CHW)
            with nc.named_scope(f"qkv{n}"):
                if n + 1 < NCH:  # prefetch next chunk while computing
                    nxt = xpool.tile([128, 8, CHW], bf16, name="xts")
                    for k in range(8):
                        qdma[k % 3].dma_start(
                            nxt[:, k, :],
                            xt_r[:, k, (n + 1) * CHW:(n + 2) * CHW])
                    xts_tiles.append(nxt)
                xts = xts_tiles[n]
                for m in range(6):
                    acc = ps.tile([128, CHW], f32, name="acc", tag="sm",
                                  bufs=4)
                    for k in range(8):
                        nc.tensor.matmul(
                            acc, wk[:, k, m * 128:(m + 1) * 128], xts[:, k, :],
                            start=(k == 0), stop=(k == 7))
                    _qkv_store(m, acc, cols)

        def emit_attn(n):
            cols = slice(n * CHW, (n + 1) * CHW)
            with nc.named_scope(f"attn{n}"):
                jmax = 4 * n + 3
                for hf in range(2):
                    pys = [
                        ps.tile([128, CHW], f32, name=f"py{sl}", tag="sm",
                                bufs=4)
                        for sl in range(2)]
                    pend = []
                    for a in range(2 * n + 2):
                        for sl in range(2):
                            h = 2 * hf + sl
                            ktp = ktp_e if sl == 0 else ktp_o
                            ss = ps.tile([128, 2, CHW], f32, name="ss",
                                         tag="big")
                            c0s = []
                            for idx in range(2):
                                j = 2 * a + idx
                                c0 = max(0, 128 * j - 512 * n)
                                c0s.append(c0)
                                nc.tensor.matmul(
                                    ss[:, idx, c0:],
                                    ktp[:, hf, j * 128:(j + 1) * 128],
                                    kqv[:, hf, n * CHW + c0:(n + 1) * CHW],
                                    start=True, stop=True)
                            if hf == 0 and a == 0 and sl == 1:
                                # v transposes for this chunk's s-tiles,
                                # hidden inside the attention PE stream
                                for t2 in range(2):
                                    jb = 4 * n + 2 * t2
                                    tp = ps.tile([128, 2, 2, 128], f32,
                                                 name="tp", tag="sm", bufs=4)
                                    for ji in range(2):
                                        for h2 in range(2):
                                            nc.tensor.transpose(
                                                tp[:, ji, h2, :],
                                                vtq[:, h2,
                                                    (jb + ji) * 128:
                                                    (jb + ji + 1) * 128],
                                                ident)
                                    for h2 in range(2):
                                        off = jb * 512 + h2 * 256
                                        anch = vsb[:, off:off + 1]
                                        dst = bass.AP(
                                            tensor=anch.tensor,
                                            offset=anch.offset,
                                            ap=[anch.ap[0], [512, 2],
                                                [192, 2], [1, 64]])
                                        nc.vector.tensor_copy(
                                            dst,
                                            tp[:, :, h2, :].rearrange(
                                                "p j (a b) -> p j a b", a=2))
                            for fn in pend:
                                fn()
                            pend = []
                            pt = ptp.tile([128, 2, CHW], bf16, name="pt")
                            # single merged exp: the c0-trimmed region holds
                            # exp(stale psum), but PV only streams [c0:]
                            nc.scalar.activation(out=pt, in_=ss, func=AF.Exp)
                            for idx in range(2):
                                j = 2 * a + idx
                                c0 = c0s[idx]
                                if j >= 4 * n:  # diagonal: 0/1 mask post-exp
                                    nc.gpsimd.tensor_tensor(
                                        out=pt[:, idx, c0:c0 + 128],
                                        in0=pt[:, idx, c0:c0 + 128],
                                        in1=amask, op=OP.mult)

                            def mk_pv(a=a, c0s=c0s, pt=pt, h=h, py=pys[sl]):
                                for idx in range(2):
                                    j = 2 * a + idx
                                    c0 = c0s[idx]
                                    nc.tensor.matmul(
                                        py[:, c0:], v_stationary(j, h),
                                        pt[:, idx, c0:],
                                        start=(j == 0), stop=(j == jmax),
                                        skip_group_check=True)
                            pend.append(mk_pv)
                    for fn in pend:
                        fn()
                    # normalization straight off PSUM: head A yT at 0:64 of
                    # pys[0] (rowsum bcast at 64:128), head B yT at 64:128 of
                    # pys[1] (rowsum bcast at 0:64). reciprocal on the
                    # complement halves, DMA partition swap, multiply.
                    rtA = rsp.tile([128, CHW], f32, name="rtA")
                    rtB = rsp.tile([128, CHW], f32, name="rtB")
                    rb = rbp.tile([128, CHW], f32, name="rb")
                    # full-partition, offset-0 APs only: the custom DVE op
                    # mis-addresses partition-offset APs. Halves 0:64 of rtA
                    # and 64:128 of rtB hold junk reciprocals, never read.
                    nc.vector.reciprocal_approx_fast(out=rtA, in_=pys[0])
                    nc.sync.dma_start(rb[0:64, :], rtA[64:128, :])
                    nc.vector.reciprocal_approx_fast(out=rtB, in_=pys[1])
                    nc.sync.dma_start(rb[64:128, :], rtB[0:64, :])
                    nc.vector.tensor_tensor(
                        out=ysb[0:64, hf, cols], in0=pys[0][0:64, :],
                        in1=rb[0:64, :], op=OP.mult)
                    nc.vector.tensor_tensor(
                        out=ysb[64:128, hf, cols], in0=pys[1][64:128, :],
                        in1=rb[64:128, :], op=OP.mult)

        def emit_proj(n):
            cols = slice(n * CHW, (n + 1) * CHW)
            with nc.named_scope(f"proj{n}"):
                for op2 in range(4):
                    acc = ps.tile([128, 2, CHW], f32, name="pacc", tag="big")
                    for cc in range(2):
                        o = 2 * op2 + cc
                        for kk in range(2):
                            nc.tensor.matmul(
                                acc[:, cc, :],
                                wp[:, kk, o * 128:(o + 1) * 128],
                                ysb[:, kk, cols],
                                start=(kk == 0), stop=(kk == 1))
                    ot = osp.tile([128, 2, CHW], bf16, name="ot")
                    if op2 % 2 == 0:
                        nc.scalar.copy(out=ot, in_=acc)
                    else:
                        nc.vector.tensor_copy(ot, acc)
                    nc.gpsimd.dma_start(
                        outp_r[:, 2 * op2:2 * op2 + 2, cols], ot)

        # ---- chunk pipeline: next QKV right after attn covers norm latency
        emit_qkv(0)
        for n in range(NCH):
            emit_attn(n)
            if n + 1 < NCH:
                emit_qkv(n + 1)
            emit_proj(n)

    nc.compile()
    return nc


def _host_inputs(x, W_kqv, b_kqv, W_proj, b_proj):
    import ml_dtypes
    bf16 = ml_dtypes.bfloat16

    x = np.asarray(x, dtype=np.float32)
    W_kqv = np.asarray(W_kqv, dtype=np.float32)
    b_kqv = np.asarray(b_kqv, dtype=np.float32)
    W_proj = np.asarray(W_proj, dtype=np.float32)

    ident = np.eye(128, dtype=np.float32)
    ss, tt = np.meshgrid(np.arange(128), np.arange(128), indexing="ij")
    amask = (ss <= tt).astype(bf16)  # 0/1 multiplicative mask

    xts = [np.ascontiguousarray(x[b].T.astype(bf16)) for b in range(B)]

    in_maps = []
    for c in range(NCORES):
        b, g = c // 4, c % 4
        heads = [4 * g + i for i in range(HPC)]
        wl = np.concatenate(
            [W_kqv[h * 192:h * 192 + 64] for h in heads]
            + [W_kqv[h * 192 + 64:h * 192 + 128] * 0.125 for h in heads]
            + [W_kqv[h * 192 + 128:h * 192 + 192] for h in heads], axis=0)
        bl = np.concatenate(
            [b_kqv[h * 192:h * 192 + 64] for h in heads]
            + [b_kqv[h * 192 + 64:h * 192 + 128] * 0.125 for h in heads]
            + [b_kqv[h * 192 + 128:h * 192 + 192] for h in heads])
        in_maps.append({
            "xt": xts[b],
            "wkqv": np.ascontiguousarray(wl.T.astype(bf16)),
            "bkq": np.ascontiguousarray(
                bl.reshape(6, 128).T.astype(np.float32)),
            "wproj": np.ascontiguousarray(
                W_proj[:, 256 * g:256 * (g + 1)].T.astype(bf16)),
            "ident": ident,
            "amask": amask,
        })
    return in_maps


def kernel(x, W_kqv, b_kqv, W_proj, b_proj):
    from concourse.bass_utils import run_bass_kernel_spmd

    if "nc" not in _cache:
        _cache["nc"] = _build_nc()
    nc = _cache["nc"]

    in_maps = _host_inputs(x, W_kqv, b_kqv, W_proj, b_proj)
    trace = bool(int(os.environ.get("KERNEL_TRACE", "0")))
    r = run_bass_kernel_spmd(nc, in_maps, core_ids=list(range(NCORES)),
                             trace=trace)
    if trace:
        _cache["last_results"] = r
        print(f"HW exec time: {r.exec_time_ns} ns")

    b_proj = np.asarray(b_proj, dtype=np.float32)
    out = np.empty((B, T, C), dtype=np.float32)
    for b in range(B):
        acc = np.zeros((C, T), dtype=np.float32)
        for g in range(4):
            acc += r.results[4 * b + g]["outp"].astype(np.float32)
        out[b] = acc.T + b_proj[None, :]
    return out


# revision 12
# speedup vs baseline: 1.1280x; 1.0168x over previous
"""Causal self-attention (B=2, T=2048, C=1024, nh=16) on 8 TRN2 NeuronCores.

Sharding: core c = 4*b + g handles batch b (2048 tokens) and head-group g
(4 heads).  Megatron-style: QKV rows and proj columns sharded by head group;
the proj partial sums (and b_proj) are reduced on the host (the
"all-reduce").

Per-core kernel, v4 — all matmul operands bf16 (PSUM accumulation stays
fp32); matmul N is capped at 512 by the ISA (s3d3_mm_num_elements):
  1. QKV projection kqvT[f,t] = Wl @ x_b.T per 512-token chunk. k-outputs
     land in zero-padded per-slot buffers (ktp_e/ktp_o) so QK matmuls
     contract K=128 with a full 128-partition moving operand.
  2. v tiles transposed on PE (f32) to [s,d] layout, packed next to shared
     all-ones column blocks (memset once) so the PV matmul also emits
     softmax row-sums pre-broadcast across the complement 64 partitions.
  3. per head, per 512-token chunk: S.T = kT.T@qT per s-tile pair into
     2-bank psum, one merged exp on ACT (bf16 out; trimmed region unread),
     0/1 causal mask on diagonal blocks (gpsimd), PV accumulation
     v_aug.T @ P.T -> yT + rowsum.
  4. normalize straight off PSUM: reciprocal_approx_fast (full-partition
     APs only — the custom DVE op mis-addresses partition offsets), DMA
     partition swap, bf16 multiplies into ysb.
  5. proj emitted per 512-chunk after the next chunk's QKV (keeps the PE
     fed while normalization completes); bf16 partials DMA'd out, host
     sums the 4 per-batch partials in fp32 and adds b_proj.

PSUM budget (8 banks): tag "big" 2-bank slots x2 bufs (attn ss / proj acc)
+ tag "sm" 1-bank slots x4 bufs (QKV accs, pys pair, v-transpose tp).
"""

import os
import numpy as np

B, T, C, NH, HD = 2, 2048, 1024, 16, 64
HPC = 4  # heads per core
NCORES = 8

_cache = {}


def _build_nc():
    from contextlib import ExitStack

    import concourse.bass as bass
    import concourse.tile as tile
    from concourse import bacc, mybir

    f32 = mybir.dt.float32
    bf16 = mybir.dt.bfloat16
    AF = mybir.ActivationFunctionType
    OP = mybir.AluOpType

    nc = bacc.Bacc("TRN2", target_bir_lowering=False, debug=False,
                   num_devices=NCORES)

    xt = nc.dram_tensor("xt", [C, T], bf16, kind="ExternalInput").ap()
    wkqv = nc.dram_tensor("wkqv", [C, 3 * HPC * HD], bf16,
                          kind="ExternalInput").ap()
    bkq = nc.dram_tensor("bkq", [128, 6], f32, kind="ExternalInput").ap()
    wproj = nc.dram_tensor("wproj", [HPC * HD, C], bf16,
                           kind="ExternalInput").ap()
    ident_d = nc.dram_tensor("ident", [128, 128], f32,
                             kind="ExternalInput").ap()
    amask_d = nc.dram_tensor("amask", [128, 128], bf16,
                             kind="ExternalInput").ap()
    outp = nc.dram_tensor("outp", [C, T], bf16, kind="ExternalOutput").ap()

    NCH = 4       # 512-wide t-chunks
    CHW = 512

    with tile.TileContext(nc) as tc, ExitStack() as ctx:
        sing = ctx.enter_context(tc.tile_pool(name="sing", bufs=1))
        xpool = ctx.enter_context(tc.tile_pool(name="xpool", bufs=2))
        ptp = ctx.enter_context(tc.tile_pool(name="ptp", bufs=4))
        rsp = ctx.enter_context(tc.tile_pool(name="rsp", bufs=4))
        rbp = ctx.enter_context(tc.tile_pool(name="rbp", bufs=2))
        osp = ctx.enter_context(tc.tile_pool(name="osp", bufs=3))
        ps = ctx.enter_context(tc.tile_pool(name="ps", bufs=2, space="PSUM"))

        # ---- resident SBUF tensors ----
        wk = sing.tile([128, 8, 768], bf16, name="wk")
        # q by head-pair (bf16); vT by head-pair in f32 for PE transposes
        kqv = sing.tile([128, 2, T], bf16, name="kqv")
        vtq = sing.tile([128, 2, T], f32, name="vtq")
        # zero-padded kT for full-K QK matmuls: even heads in rows 0:64 of
        # ktp_e (rows 64:128 zero), odd heads in rows 64:128 of ktp_o
        ktp_e = sing.tile([128, 2, T], bf16, name="ktp_e")
        ktp_o = sing.tile([128, 2, T], bf16, name="ktp_o")
        # vsb: 32 blocks of [v_A(64) | ones(128) | v_B(64)]
        vsb = sing.tile([128, 32 * 256], bf16, name="vsb")
        ysb = sing.tile([128, 2, T], bf16, name="ysb")
        wp = sing.tile([128, 2, C], bf16, name="wp")
        bkq_s = sing.tile([128, 6], f32, name="bkq_s")
        ident = sing.tile([128, 128], f32, name="ident")
        amask = sing.tile([128, 128], bf16, name="amask")

        nc.vector.memset(ktp_e[64:128, :, :], 0.0)
        nc.vector.memset(ktp_o[0:64, :, :], 0.0)
        # shared all-ones mid columns of every vsb block: PV matmul emits
        # rowsum broadcast over the complement 64 partitions of each yT
        vanchor = vsb[:, 64:65]
        vones = bass.AP(tensor=vanchor.tensor, offset=vanchor.offset,
                        ap=[vanchor.ap[0], [256, 32], [1, 128]])
        nc.vector.memset(vones, 1.0)

        xt_r = xt.rearrange("(kk p) t -> p kk t", p=128)
        wkqv_r = wkqv.rearrange("(kk p) f -> p kk f", p=128)
        outp_r = outp.rearrange("(a p) t -> p a t", p=128)

        qdma = [nc.scalar, nc.gpsimd, nc.sync]
        xts0 = xpool.tile([128, 8, CHW], bf16, name="xts")
        # cold start: x chunk 0 and the m=0..2 weight columns first, so the
        # early QKV m-groups start as soon as possible; rest follows.
        for k in range(8):
            qdma[k % 3].dma_start(xts0[:, k, :], xt_r[:, k, 0:CHW])
            qdma[k % 3].dma_start(wk[:, k, 0:384], wkqv_r[:, k, 0:384])
        nc.scalar.dma_start(bkq_s, bkq)
        nc.gpsimd.dma_start(ident, ident_d)
        nc.sync.dma_start(amask, amask_d)
        for k in range(8):
            qdma[k % 3].dma_start(wk[:, k, 384:768], wkqv_r[:, k, 384:768])
        nc.sync.dma_start(wp, wproj.rearrange("(kk p) f -> p kk f", p=128))

        def v_stationary(j, h):
            """[128,128] AP: even slot -> [v_A|ones64], odd -> [ones64|v_B]."""
            hf, sl = h // 2, h % 2
            off = (j * 2 + hf) * 256 + 128 * sl
            return vsb[:, off:off + 128]

        xts_tiles = [xts0]

        def _qkv_store(m, acc, cols):
            if m < 2:  # kT: split into zero-padded buffers
                nc.vector.tensor_scalar_add(
                    out=ktp_e[0:64, m, cols], in0=acc[0:64, :],
                    scalar1=bkq_s[0:64, m:m + 1])
                nc.vector.tensor_scalar_add(
                    out=ktp_o[64:128, m, cols], in0=acc[64:128, :],
                    scalar1=bkq_s[64:128, m:m + 1])
            elif m < 4:
                nc.vector.tensor_scalar_add(
                    out=kqv[:, m - 2, cols], in0=acc,
                    scalar1=bkq_s[:, m:m + 1])
            else:
                nc.vector.tensor_scalar_add(
                    out=vtq[:, m - 4, cols], in0=acc,
                    scalar1=bkq_s[:, m:m + 1])

        def emit_qkv(n):
            # m: 0,1 -> kT (head pairs), 2,3 -> q, 4,5 -> v
            cols = slice(n * CHW, (n + 1) * CHW)
            with nc.named_scope(f"qkv{n}"):
                if n + 1 < NCH:  # prefetch next chunk while computing
                    nxt = xpool.tile([128, 8, CHW], bf16, name="xts")
                    for k in range(8):
                        qdma[k % 3].dma_start(
                            nxt[:, k, :],
                            xt_r[:, k, (n + 1) * CHW:(n + 2) * CHW])
                    xts_tiles.append(nxt)
                xts = xts_tiles[n]
                for m in range(6):
                    acc = ps.tile([128, CHW], f32, name="acc", tag="sm",
                                  bufs=4)
                    for k in range(8):
                        nc.tensor.matmul(
                            acc, wk[:, k, m * 128:(m + 1) * 128], xts[:, k, :],
                            start=(k == 0), stop=(k == 7))
                    _qkv_store(m, acc, cols)

        def emit_attn(n):
            cols = slice(n * CHW, (n + 1) * CHW)
            with nc.named_scope(f"attn{n}"):
                jmax = 4 * n + 3
                for hf in range(2):
                    pys = [
                        ps.tile([128, CHW], f32, name=f"py{sl}", tag="sm",
                                bufs=4)
                        for sl in range(2)]
                    pend = []
                    for a in range(2 * n + 2):
                        for sl in range(2):
                            h = 2 * hf + sl
                            ktp = ktp_e if sl == 0 else ktp_o
                            ss = ps.tile([128, 2, CHW], f32, name="ss",
                                         tag="big")
                            c0s = []
                            for idx in range(2):
                                j = 2 * a + idx
                                c0 = max(0, 128 * j - 512 * n)
                                c0s.append(c0)
                                nc.tensor.matmul(
                                    ss[:, idx, c0:],
                                    ktp[:, hf, j * 128:(j + 1) * 128],
                                    kqv[:, hf, n * CHW + c0:(n + 1) * CHW],
                                    start=True, stop=True)
                            if hf == 0 and a == 0 and sl == 1:
                                # v transposes for this chunk's s-tiles,
                                # hidden inside the attention PE stream
                                for t2 in range(2):
                                    jb = 4 * n + 2 * t2
                                    tp = ps.tile([128, 2, 2, 128], f32,
                                                 name="tp", tag="sm", bufs=4)
                                    for ji in range(2):
                                        for h2 in range(2):
                                            nc.tensor.transpose(
                                                tp[:, ji, h2, :],
                                                vtq[:, h2,
                                                    (jb + ji) * 128:
                                                    (jb + ji + 1) * 128],
                                                ident)
                                    for h2 in range(2):
                                        off = jb * 512 + h2 * 256
                                        anch = vsb[:, off:off + 1]
                                        dst = bass.AP(
                                            tensor=anch.tensor,
                                            offset=anch.offset,
                                            ap=[anch.ap[0], [512, 2],
                                                [192, 2], [1, 64]])
                                        nc.vector.tensor_copy(
                                            dst,
                                            tp[:, :, h2, :].rearrange(
                                                "p j (a b) -> p j a b", a=2))
                            for fn in pend:
                                fn()
                            pend = []
                            pt = ptp.tile([128, 2, CHW], bf16, name="pt")
                            # single merged exp: the c0-trimmed region holds
                            # exp(stale psum), but PV only streams [c0:]
                            nc.scalar.activation(out=pt, in_=ss, func=AF.Exp)
                            for idx in range(2):
                                j = 2 * a + idx
                                c0 = c0s[idx]
                                if j >= 4 * n:  # diagonal: 0/1 mask post-exp
                                    nc.gpsimd.tensor_tensor(
                                        out=pt[:, idx, c0:c0 + 128],
                                        in0=pt[:, idx, c0:c0 + 128],
                                        in1=amask, op=OP.mult)

                            def mk_pv(a=a, c0s=c0s, pt=pt, h=h, py=pys[sl]):
                                for idx in range(2):
                                    j = 2 * a + idx
                                    c0 = c0s[idx]
                                    nc.tensor.matmul(
                                        py[:, c0:], v_stationary(j, h),
                                        pt[:, idx, c0:],
                                        start=(j == 0), stop=(j == jmax),
                                        skip_group_check=True)
                            pend.append(mk_pv)
                    for fn in pend:
                        fn()
                    # normalization straight off PSUM: head A yT at 0:64 of
                    # pys[0] (rowsum bcast at 64:128), head B yT at 64:128 of
                    # pys[1] (rowsum bcast at 0:64). reciprocal on the
                    # complement halves, DMA partition swap, multiply.
                    rtA = rsp.tile([128, CHW], f32, name="rtA")
                    rtB = rsp.tile([128, CHW], f32, name="rtB")
                    rb = rbp.tile([128, CHW], f32, name="rb")
                    # full-partition, offset-0 APs only: the custom DVE op
                    # mis-addresses partition-offset APs. Halves 0:64 of rtA
                    # and 64:128 of rtB hold junk reciprocals, never read.
                    nc.vector.reciprocal_approx_fast(out=rtA, in_=pys[0])
                    nc.sync.dma_start(rb[0:64, :], rtA[64:128, :])
                    nc.vector.reciprocal_approx_fast(out=rtB, in_=pys[1])
                    nc.sync.dma_start(rb[64:128, :], rtB[0:64, :])
                    nc.vector.tensor_tensor(
                        out=ysb[0:64, hf, cols], in0=pys[0][0:64, :],
                        in1=rb[0:64, :], op=OP.mult)
                    nc.vector.tensor_tensor(
                        out=ysb[64:128, hf, cols], in0=pys[1][64:128, :],
                        in1=rb[64:128, :], op=OP.mult)

        def emit_proj(n, oprange=range(4)):
            cols = slice(n * CHW, (n + 1) * CHW)
            with nc.named_scope(f"proj{n}"):
                for op2 in oprange:
                    acc = ps.tile([128, 2, CHW], f32, name="pacc", tag="big")
                    for cc in range(2):
                        o = 2 * op2 + cc
                        for kk in range(2):
                            nc.tensor.matmul(
                                acc[:, cc, :],
                                wp[:, kk, o * 128:(o + 1) * 128],
                                ysb[:, kk, cols],
                                start=(kk == 0), stop=(kk == 1))
                    ot = osp.tile([128, 2, CHW], bf16, name="ot")
                    if op2 % 2 == 0:
                        nc.scalar.copy(out=ot, in_=acc)
                    else:
                        nc.vector.tensor_copy(ot, acc)
                    (nc.gpsimd if op2 % 2 == 0 else nc.sync).dma_start(
                        outp_r[:, 2 * op2:2 * op2 + 2, cols], ot)

        # ---- chunk pipeline: next QKV right after attn covers norm
        # latency; half of proj2 is held back to keep the PE busy (and at
        # full p-state) while attn3's final normalization completes
        emit_qkv(0)
        emit_attn(0)
        emit_qkv(1)
        emit_proj(0)
        emit_attn(1)
        emit_qkv(2)
        emit_proj(1)
        emit_attn(2)
        emit_qkv(3)
        emit_proj(2, range(0, 2))
        emit_attn(3)
        emit_proj(2, range(2, 4))
        emit_proj(3)

    nc.compile()
    return nc


def _host_inputs(x, W_kqv, b_kqv, W_proj, b_proj):
    import ml_dtypes
    bf16 = ml_dtypes.bfloat16

    x = np.asarray(x, dtype=np.float32)
    W_kqv = np.asarray(W_kqv, dtype=np.float32)
    b_kqv = np.asarray(b_kqv, dtype=np.float32)
    W_proj = np.asarray(W_proj, dtype=np.float32)

    ident = np.eye(128, dtype=np.float32)
    ss, tt = np.meshgrid(np.arange(128), np.arange(128), indexing="ij")
    amask = (ss <= tt).astype(bf16)  # 0/1 multiplicative mask

    xts = [np.ascontiguousarray(x[b].T.astype(bf16)) for b in range(B)]

    in_maps = []
    for c in range(NCORES):
        b, g = c // 4, c % 4
        heads = [4 * g + i for i in range(HPC)]
        wl = np.concatenate(
            [W_kqv[h * 192:h * 192 + 64] for h in heads]
            + [W_kqv[h * 192 + 64:h * 192 + 128] * 0.125 for h in heads]
            + [W_kqv[h * 192 + 128:h * 192 + 192] for h in heads], axis=0)
        bl = np.concatenate(
            [b_kqv[h * 192:h * 192 + 64] for h in heads]
            + [b_kqv[h * 192 + 64:h * 192 + 128] * 0.125 for h in heads]
            + [b_kqv[h * 192 + 128:h * 192 + 192] for h in heads])
        in_maps.append({
            "xt": xts[b],
            "wkqv": np.ascontiguousarray(wl.T.astype(bf16)),
            "bkq": np.ascontiguousarray(
                bl.reshape(6, 128).T.astype(np.float32)),
            "wproj": np.ascontiguousarray(
                W_proj[:, 256 * g:256 * (g + 1)].T.astype(bf16)),
            "ident": ident,
            "amask": amask,
        })
    return in_maps


def kernel(x, W_kqv, b_kqv, W_proj, b_proj):
    from concourse.bass_utils import run_bass_kernel_spmd

    if "nc" not in _cache:
        _cache["nc"] = _build_nc()
    nc = _cache["nc"]

    in_maps = _host_inputs(x, W_kqv, b_kqv, W_proj, b_proj)
    trace = bool(int(os.environ.get("KERNEL_TRACE", "0")))
    r = run_bass_kernel_spmd(nc, in_maps, core_ids=list(range(NCORES)),
                             trace=trace)
    if trace:
        _cache["last_results"] = r
        print(f"HW exec time: {r.exec_time_ns} ns")

    b_proj = np.asarray(b_proj, dtype=np.float32)
    out = np.empty((B, T, C), dtype=np.float32)
    for b in range(B):
        acc = np.zeros((C, T), dtype=np.float32)
        for g in range(4):
            acc += r.results[4 * b + g]["outp"].astype(np.float32)
        out[b] = acc.T + b_proj[None, :]
    return out
